# revision 24
# baseline (speedup 1.0000x reference)
"""Trainium2 Bass kernel for causal multi-head attention.

Problem: B=2, S=2048, D=1024, H=16 heads (DH=64), causal, fp32 reference.

Sharding over 8 NeuronCores: core c handles batch b = c//4 and head group
g = c%4 (4 heads each).  Wq/Wk/Wv are split column-wise (by output head),
Wo row-wise; per-core partial outputs are summed on the host (row-parallel
Wo => partial sums), then bo is added.

Per-core device kernel (matmul operands bf16, fp32 PSUM accumulation):
  qT/kT = W @ xT            (64, 2048) per head, head pairs stacked on 128 parts
  v     = x @ WvT           (2048, 256)
  S^T   = k q^T             scores transposed: (s_k, s_q) tiles; diagonal
                            blocks shrunk to the causally-valid q range
  P     = exp(0.125 * S^T)  ScalarE (only exp runs there); the 128-wide
                            triangle at the diagonal masked via DVE mul
  ctx^T = v^T P             accumulated over k tiles in PSUM
  sums  = onescol^T P       quad accumulators at partitions 0/32/64/96
  denominators: DVE evac -> PE head-sum matmul -> DVE reciprocal ->
                GpSimd partition broadcasts -> DVE normalize multiply,
                all riding the next q tile's k loop as fine-grained work
  out   = ctx @ WoT         (2048, 1024) fp32 partial, riding the next loop

All non-attention work (projections of the next q tile, output projection
of the previous one, the normalize chain) is queued as "riders" that are
drained at a fixed cadence inside the attention k loops so PE/DVE/ACT all
stay busy; ScalarE's exp paces the steady state.
"""

import sys

sys.path.insert(0, "/opt/trn_rl_repo")

import numpy as np
import ml_dtypes

import concourse.bass as bass
import concourse.bacc as bacc
import concourse.mybir as mybir
import concourse.tile as tile
from concourse.bass_utils import run_bass_kernel_spmd

BF16 = mybir.dt.bfloat16
F32 = mybir.dt.float32
AF = mybir.ActivationFunctionType

B, S, D, H = 2, 2048, 1024, 16
DH = D // H            # 64
NCORES = 8
NH = 4                 # heads per core
DL = NH * DH           # 256 local head dims per core
KD = D // 128          # 8 contraction chunks for projections
SQ = S // 512          # 4 q tiles of 512
ST = S // 128          # 16 s tiles of 128
SCALE = DH ** -0.5     # 0.125
LAG = 3                # ctx trails scores by LAG k-tiles

_NC = None

TRACE = False
LAST_RESULTS = None
DEBUG = False


def _build_nc():
    nc = bacc.Bacc("TRN2", target_bir_lowering=False, debug=False,
                   num_devices=NCORES)

    xT_d = nc.dram_tensor("xT", [D, S], BF16, kind="ExternalInput")
    wq_d = nc.dram_tensor("wqT", [D, DL], BF16, kind="ExternalInput")
    wk_d = nc.dram_tensor("wkT", [D, DL], BF16, kind="ExternalInput")
    wv_d = nc.dram_tensor("wvT", [D, DL], BF16, kind="ExternalInput")
    wo_d = nc.dram_tensor("woT", [DL, D], BF16, kind="ExternalInput")
    # [128, 2, 128] lower-triangle (q >= k) mask, duplicated per head half
    tri_d = nc.dram_tensor("tri", [128, 256], BF16, kind="ExternalInput")
    # head-sum selector: col h picks quad rows {64*h_parity...}; see below
    sel128_d = nc.dram_tensor("sel128", [128, 2], F32, kind="ExternalInput")
    # ones in column 0, zeros elsewhere (M=32 so quad rows are zero-filled)
    onescol_d = nc.dram_tensor("onescol", [128, 32], BF16, kind="ExternalInput")
    # broadcast selector: row h -> ones at cols [64h, 64h+64)
    sel2_d = nc.dram_tensor("sel2", [2, 128], F32, kind="ExternalInput")
    out_d = nc.dram_tensor("out", [S, D], F32, kind="ExternalOutput")
    dbg = {}
    if DEBUG:
        for j in (1, 2):
            for p in (0, 1):
                dbg[f"q4_{j}_{p}"] = nc.dram_tensor(
                    f"dbg_q4_{j}_{p}", [128, 512], F32, kind="ExternalOutput")
                dbg[f"inv2_{j}_{p}"] = nc.dram_tensor(
                    f"dbg_inv2_{j}_{p}", [2, 512], F32, kind="ExternalOutput")
                dbg[f"invb_{j}_{p}"] = nc.dram_tensor(
                    f"dbg_invb_{j}_{p}", [128, 512], F32, kind="ExternalOutput")
                dbg[f"ctxT_{j}_{p}"] = nc.dram_tensor(
                    f"dbg_ctxT_{j}_{p}", [128, 512], BF16, kind="ExternalOutput")
                dbg[f"ex_{j}_{p}"] = nc.dram_tensor(
                    f"dbg_ex_{j}_{p}", [128, 1024], BF16, kind="ExternalOutput")

    with tile.TileContext(nc) as tc:
        with (
            tc.tile_pool(name="consts", bufs=1) as consts,
            tc.tile_pool(name="xpool", bufs=1) as xpool,
            tc.tile_pool(name="qkpool", bufs=1) as qkpool,
            tc.tile_pool(name="vpool", bufs=1) as vpool,
            tc.tile_pool(name="cpool", bufs=1) as cpool,
            tc.tile_pool(name="exppool", bufs=6) as exppool,
            tc.tile_pool(name="smallpool", bufs=2) as smallpool,
            tc.tile_pool(name="outpool", bufs=6) as outpool,
            tc.tile_pool(name="spsum", bufs=2, space="PSUM") as spsum,
            tc.tile_pool(name="ctxpsum", bufs=2, space="PSUM") as ctxpsum,
            tc.tile_pool(name="mpsum", bufs=1, space="PSUM") as mpsum,
            tc.tile_pool(name="rpsum", bufs=1, space="PSUM") as rpsum,
        ):
            # ---- constants (order matters: q/k weights first, then sq0 x) ----
            wq_sb = consts.tile([128, KD, DL], BF16)
            wk_sb = consts.tile([128, KD, DL], BF16)
            wv_sb = consts.tile([128, KD, DL], BF16)
            wo_sb = consts.tile([128, 2, D], BF16)
            tri_sb = consts.tile([128, 256], BF16)
            sel128_sb = consts.tile([128, 2], F32)
            onescol_sb = consts.tile([128, 32], BF16)
            sel2_sb = consts.tile([2, 128], F32)
            zb = consts.tile([128, 1], F32)

            nc.sync.dma_start(wq_sb[:], wq_d.rearrange("(kd p) j -> p kd j", p=128))
            nc.sync.dma_start(wk_sb[:], wk_d.rearrange("(kd p) j -> p kd j", p=128))

            # ---- persistent activations ----
            xt = [
                [xpool.tile([128, 512], BF16, name=f"xt_{kd}_{sq}",
                            tag=f"xt_{kd}_{sq}") for sq in range(SQ)]
                for kd in range(KD)
            ]
            qT = [
                [qkpool.tile([128, 512], BF16, name=f"qT_{m2}_{sq}",
                             tag=f"qT_{m2}_{sq}") for sq in range(SQ)]
                for m2 in range(2)
            ]
            kT = [
                [qkpool.tile([128, 512], BF16, name=f"kT_{m2}_{sq}",
                             tag=f"kT_{m2}_{sq}") for sq in range(SQ)]
                for m2 in range(2)
            ]
            vt = [
                vpool.tile([128, NH, DH], BF16, name=f"v_{st}", tag=f"v_{st}")
                for st in range(ST)
            ]
            ctxT = [
                [cpool.tile([128, 512], BF16, name=f"ctxT_{kc}_{sq}",
                            tag=f"ctxT_{kc}_{sq}") for sq in range(SQ)]
                for kc in range(2)
            ]

            xT_r = xT_d.rearrange("(kd p) s -> kd p s", p=128)
            # x goes over the (otherwise idle) GpSimd SWDGE queue so the
            # weight loads and output stores on the sync HW queue don't
            # serialize behind it; sq=0 first so projections start early
            for sq in range(SQ):
                for kd in range(KD):
                    nc.gpsimd.dma_start(
                        xt[kd][sq][:], xT_r[kd, :, sq * 512:(sq + 1) * 512]
                    )
            nc.sync.dma_start(tri_sb[:], tri_d[:])
            nc.sync.dma_start(sel128_sb[:], sel128_d[:])
            nc.sync.dma_start(onescol_sb[:], onescol_d[:])
            nc.sync.dma_start(sel2_sb[:], sel2_d[:])
            nc.vector.memset(zb[:], 0.0)
            nc.sync.dma_start(
                wv_sb[:], wv_d.rearrange("(kd p) j -> p kd j", p=128))
            nc.sync.dma_start(
                wo_sb[:], wo_d.rearrange("(kc p) o -> p kc o", p=128))

            # ---- work emitters ----
            def emit_qk_group(sq, which, m2):
                w_sb = wq_sb if which == "q" else wk_sb
                dst = qT if which == "q" else kT
                ps = rpsum.tile([128, 512], F32, name="r_ps", tag="r")
                for kd in range(KD):
                    nc.tensor.matmul(
                        ps[:],
                        w_sb[:, kd, m2 * 128:(m2 + 1) * 128],
                        xt[kd][sq][:],
                        start=(kd == 0),
                        stop=(kd == KD - 1),
                    )
                nc.vector.tensor_copy(dst[m2][sq][:], ps[:])

            def qk_half_riders(sq, which, m2):
                # one q/k projection group split into two riders (4 of the 8
                # contraction chunks each) so a single slot's PE burst stays
                # short; the PSUM accumulator is shared via the closure
                w_sb = wq_sb if which == "q" else wk_sb
                dst = qT if which == "q" else kT
                st = {}

                def h1():
                    ps = rpsum.tile([128, 512], F32, name="r_ps", tag="r")
                    for kd in range(4):
                        nc.tensor.matmul(
                            ps[:],
                            w_sb[:, kd, m2 * 128:(m2 + 1) * 128],
                            xt[kd][sq][:],
                            start=(kd == 0),
                            stop=False,
                        )
                    st["ps"] = ps

                def h2():
                    ps = st["ps"]
                    for kd in range(4, KD):
                        nc.tensor.matmul(
                            ps[:],
                            w_sb[:, kd, m2 * 128:(m2 + 1) * 128],
                            xt[kd][sq][:],
                            start=False,
                            stop=(kd == KD - 1),
                        )
                    nc.vector.tensor_copy(dst[m2][sq][:], ps[:])

                return [h1, h2]

            def emit_v_group(sq, sti):
                st = sq * 4 + sti
                ps = rpsum.tile([128, DL], F32, name="r_ps", tag="r")
                for kd in range(KD):
                    nc.tensor.matmul(
                        ps[:],
                        xt[kd][sq][:, sti * 128:(sti + 1) * 128],
                        wv_sb[:, kd, :],
                        start=(kd == 0),
                        stop=(kd == KD - 1),
                    )
                nc.vector.tensor_copy(
                    vt[st][:].rearrange("p h d -> p (h d)"), ps[:]
                )

            def emit_wo(j, sti, ot):
                st = j * 4 + sti
                o_ps = rpsum.tile([128, 512], F32, name="r_ps", tag="r")
                for kc in range(2):
                    nc.tensor.matmul(
                        o_ps[:],
                        ctxT[kc][j][:, sti * 128:(sti + 1) * 128],
                        wo_sb[:, kc, ot * 512:(ot + 1) * 512],
                        start=(kc == 0),
                        stop=(kc == 1),
                    )
                ob = outpool.tile([128, 512], F32, name="ob", tag="ob")
                nc.vector.tensor_copy(ob[:], o_ps[:])
                nc.sync.dma_start(
                    out_d[st * 128:(st + 1) * 128, ot * 512:(ot + 1) * 512],
                    ob[:],
                )

            # normalize chain state per (j, p): set at loop end, consumed by
            # riders in the following loop
            def make_chain(j, p, q4_sb, c_ps):
                st = {}

                def s_mm1():
                    den2 = rpsum.tile([2, 512], F32, name="den2", tag="r")
                    nc.tensor.matmul(den2[:], sel128_sb[:], q4_sb[:],
                                     start=True, stop=True)
                    st["den2"] = den2

                def s_recip():
                    inv2 = smallpool.tile([2, 512], F32, name="inv2",
                                          tag="inv2")
                    nc.vector.reciprocal_approx_fast(inv2[:], st["den2"][:])
                    st["inv2"] = inv2
                    if DEBUG and f"inv2_{j}_{p}" in dbg:
                        nc.sync.dma_start(dbg[f"inv2_{j}_{p}"][:], inv2[:])

                def s_bcast():
                    inv_ps = rpsum.tile([128, 512], F32, name="inv_ps",
                                        tag="r")
                    nc.tensor.matmul(inv_ps[:], sel2_sb[:], st["inv2"][:],
                                     start=True, stop=True)
                    invb = smallpool.tile([128, 512], F32, name="invb",
                                          tag="invb")
                    nc.vector.tensor_copy(invb[:], inv_ps[:])
                    st["invb"] = invb
                    if DEBUG and f"invb_{j}_{p}" in dbg:
                        nc.sync.dma_start(dbg[f"invb_{j}_{p}"][:], invb[:])

                def s_mul():
                    nc.vector.tensor_mul(ctxT[p][j][:], c_ps[:],
                                         st["invb"][:])
                    if DEBUG and f"ctxT_{j}_{p}" in dbg:
                        nc.sync.dma_start(dbg[f"ctxT_{j}_{p}"][:],
                                          ctxT[p][j][:])

                return [s_mm1, s_recip, s_bcast, s_mul]

            # ---- attention loop for one (j, p) ----
            def attention(j, p, early, bulk, after=[]):
                nkt = 4 * j + 4
                # `early` chain riders run one-per-slot from slot 0 (they
                # recycle the ctx/m PSUM tiles); `bulk` riders (independent of
                # the chain) spread evenly over the loop; `after` riders (the
                # previous q tile's Wo, which READS what the chain writes)
                # must be emitted strictly after the last chain rider.
                E = len(early)
                L = len(bulk)
                A = len(after)
                rem = max(1, nkt - E)
                sched = [
                    ([early[kt]] if kt < E else [])
                    + bulk[(kt * L) // nkt:((kt + 1) * L) // nkt]
                    + (after[((kt - E) * A) // rem:((kt - E + 1) * A) // rem]
                       if kt >= E else [])
                    for kt in range(nkt)
                ]
                c_ps = ctxpsum.tile([128, 512], F32, name="c_ps", tag="ctx")
                m_ps = mpsum.tile([128, 512], F32, name="m_ps", tag="m")
                if j == 0:
                    # odd-kt quad rows start at o=1 (cols 128..512); zero the
                    # never-written head columns so the head-sum matmul reads
                    # defined data
                    nc.vector.memset(m_ps[64:128, 0:128], 0.0)
                exs = {}

                def emit_scores_exp(kt):
                    o = kt - 4 * j
                    q0 = 128 * o if o > 0 else 0
                    s_ps = spsum.tile([128, 1024], F32, name="s_ps", tag="s")
                    for i2 in range(2):
                        hr = i2 * 64
                        nc.tensor.matmul(
                            s_ps[:, i2 * 512 + q0:(i2 + 1) * 512],
                            kT[p][kt // 4][hr:hr + 64,
                                           (kt % 4) * 128:(kt % 4 + 1) * 128],
                            qT[p][j][hr:hr + 64, q0:512],
                            start=True,
                            stop=True,
                        )
                    ex = exppool.tile([128, 1024], BF16, name="ex", tag="ex")
                    if q0 == 0:
                        nc.scalar.activation(
                            ex[:], s_ps[:], AF.Exp, bias=zb[:], scale=SCALE
                        )
                    else:
                        e3 = ex[:].rearrange("p (h q) -> p h q", h=2)
                        s3 = s_ps[:].rearrange("p (h q) -> p h q", h=2)
                        nc.scalar.activation(
                            e3[:, :, q0:512], s3[:, :, q0:512], AF.Exp,
                            bias=zb[:], scale=SCALE
                        )
                    if o >= 0:
                        # triangle mask on the 128-wide diagonal window
                        e3 = ex[:].rearrange("p (h q) -> p h q", h=2)
                        t3 = tri_sb[:].rearrange("p (h q) -> p h q", h=2)
                        nc.vector.tensor_mul(
                            e3[:, :, q0:q0 + 128], e3[:, :, q0:q0 + 128],
                            t3[:]
                        )
                    exs[kt] = ex
                    if DEBUG and kt == 4 * j + 1 and f"ex_{j}_{p}" in dbg:
                        nc.sync.dma_start(dbg[f"ex_{j}_{p}"][:], ex[:])

                def emit_ctx(kt):
                    o = kt - 4 * j
                    q0 = 128 * o if o > 0 else 0
                    ex = exs[kt]
                    e3 = ex[:].rearrange("p (h q) -> p h q", h=2)
                    for i2 in range(2):
                        nc.tensor.matmul(
                            c_ps[64 * i2:64 * i2 + DH, q0:512],
                            vt[kt][:, 2 * p + i2, :],
                            e3[:, i2, q0:512],
                            start=(kt == 0),
                            stop=(kt == nkt - 1),
                            tile_position=(0, 64 * i2),
                        )
                    if kt % 2 == 1:
                        op = kt - 1 - 4 * j
                        qp = 128 * op if op > 0 else 0
                        exprev = exs.pop(kt - 1)
                        ep3 = exprev[:].rearrange("p (h q) -> p h q", h=2)
                        for idx, (eq3, i2, qq0) in enumerate(
                            [(ep3, 0, qp), (ep3, 1, qp),
                             (e3, 0, q0), (e3, 1, q0)]
                        ):
                            pos = 32 * idx
                            nc.tensor.matmul(
                                m_ps[pos:pos + 32, qq0:512],
                                onescol_sb[:],
                                eq3[:, i2, qq0:512],
                                start=(kt == 1),
                                stop=(kt == nkt - 1),
                                tile_position=(0, pos),
                            )

                for kt in range(nkt):
                    emit_scores_exp(kt)
                    if kt >= LAG:
                        emit_ctx(kt - LAG)
                    for r in sched[kt]:
                        r()
                for kt in range(max(0, nkt - LAG), nkt):
                    emit_ctx(kt)
                exs.clear()

                # evacuate the quad sums promptly (frees m_ps for the next
                # loop); the rest of the chain rides the next loop
                q4_sb = smallpool.tile([128, 512], F32, name="q4", tag="q4")
                nc.vector.tensor_copy(q4_sb[:], m_ps[:])
                if DEBUG and f"q4_{j}_{p}" in dbg:
                    nc.sync.dma_start(dbg[f"q4_{j}_{p}"][:], q4_sb[:])
                return make_chain(j, p, q4_sb, c_ps)

            # ---- prologue: just enough to start (j=0, p=0) ----
            emit_qk_group(0, "q", 0)
            emit_qk_group(0, "k", 0)
            emit_v_group(0, 0)
            emit_v_group(0, 1)

            def qkv_riders(sq):
                return (
                    qk_half_riders(sq, "q", 0) + qk_half_riders(sq, "k", 0)
                    + qk_half_riders(sq, "q", 1) + qk_half_riders(sq, "k", 1)
                    + [
                        lambda s=sq: emit_v_group(s, 0),
                        lambda s=sq: emit_v_group(s, 1),
                        lambda s=sq: emit_v_group(s, 2),
                        lambda s=sq: emit_v_group(s, 3),
                    ]
                )

            def wo_riders(j):
                return [
                    (lambda jj=j, s=sti, o=ot: emit_wo(jj, s, o))
                    for sti in range(4) for ot in range(2)
                ]

            # ---- main loops ----
            chain = {}
            chain[(0, 0)] = attention(
                0, 0, [],
                [lambda: emit_qk_group(0, "q", 1),
                 lambda: emit_qk_group(0, "k", 1),
                 lambda: emit_v_group(0, 2),
                 lambda: emit_v_group(0, 3)],
            )
            chain[(0, 1)] = attention(0, 1, chain[(0, 0)], qkv_riders(1))
            # sq=3's k/v projections ride j=3's own (ScalarE-bound) loops —
            # they are only consumed from kt=12 — so PE-bound j=2 stays lean
            bulk_j = {
                1: qkv_riders(2),
                2: qk_half_riders(3, "q", 0) + qk_half_riders(3, "q", 1),
                3: (qk_half_riders(3, "k", 0) + qk_half_riders(3, "k", 1)
                    + [lambda t=sti: emit_v_group(3, t) for sti in range(4)]),
            }
            for j in range(1, SQ):
                chain[(j, 0)] = attention(j, 0, chain[(j - 1, 1)], bulk_j[j],
                                          after=wo_riders(j - 1))
                chain[(j, 1)] = attention(j, 1, chain[(j, 0)], [])

            # ---- epilogue ----
            for step in chain[(SQ - 1, 1)]:
                step()
            for sti in range(4):
                for ot in range(2):
                    emit_wo(SQ - 1, sti, ot)

    nc.compile()
    return nc


def _get_nc():
    global _NC
    if _NC is None:
        _NC = _build_nc()
    return _NC


def _bf16(a):
    return np.ascontiguousarray(a).astype(ml_dtypes.bfloat16)


def kernel(x, Wq, Wk, Wv, Wo, bo):
    global LAST_RESULTS
    x = np.asarray(x, dtype=np.float32)
    Wq = np.asarray(Wq, dtype=np.float32)
    Wk = np.asarray(Wk, dtype=np.float32)
    Wv = np.asarray(Wv, dtype=np.float32)
    Wo = np.asarray(Wo, dtype=np.float32)
    bo = np.asarray(bo, dtype=np.float32)

    xT = [_bf16(x[b].T) for b in range(B)]          # (D, S)
    WqT = np.ascontiguousarray(Wq.T)                # (D, D): col slice = head rows
    WkT = np.ascontiguousarray(Wk.T)
    WvT = np.ascontiguousarray(Wv.T)
    WoT = np.ascontiguousarray(Wo.T)                # (D, D): row slice = ctx dims

    kk = np.arange(128)[:, None]
    cc = np.arange(128)[None, :]
    tri1 = (cc >= kk).astype(np.float32)            # (128, 128)
    tri = np.concatenate([tri1, tri1], axis=1).astype(ml_dtypes.bfloat16)

    sel128 = np.zeros((128, 2), dtype=np.float32)
    sel128[0, 0] = sel128[64, 0] = 1.0              # head 0: even+odd rows
    sel128[32, 1] = sel128[96, 1] = 1.0             # head 1
    onescol = np.zeros((128, 32), dtype=np.float32)
    onescol[:, 0] = 1.0
    onescol = onescol.astype(ml_dtypes.bfloat16)
    sel2 = np.zeros((2, 128), dtype=np.float32)
    sel2[0, 0:64] = 1.0
    sel2[1, 64:128] = 1.0

    in_maps = []
    for c in range(NCORES):
        b, g = divmod(c, 4)
        sl = slice(g * DL, (g + 1) * DL)
        in_maps.append(
            {
                "xT": xT[b],
                "wqT": _bf16(WqT[:, sl]),
                "wkT": _bf16(WkT[:, sl]),
                "wvT": _bf16(WvT[:, sl]),
                "woT": _bf16(WoT[sl, :]),
                "tri": tri,
                "sel128": sel128,
                "onescol": onescol,
                "sel2": sel2,
            }
        )

    nc = _get_nc()
    results = run_bass_kernel_spmd(
        nc, in_maps, core_ids=list(range(NCORES)), trace=TRACE
    )
    LAST_RESULTS = results

    out = np.zeros((B, S, D), dtype=np.float32)
    for c in range(NCORES):
        out[c // 4] += results.results[c]["out"]
    out += bo[None, None, :]
    return out


# revision 26
# speedup vs baseline: 1.0092x; 1.0092x over previous
"""Trainium2 Bass kernel for causal multi-head attention.

Problem: B=2, S=2048, D=1024, H=16 heads (DH=64), causal, fp32 reference.

Sharding over 8 NeuronCores: core c handles batch b = c//4 and head group
g = c%4 (4 heads each).  Wq/Wk/Wv are split column-wise (by output head),
Wo row-wise; per-core partial outputs are summed on the host (row-parallel
Wo => partial sums), then bo is added.

Per-core device kernel (matmul operands bf16, fp32 PSUM accumulation):
  qT/kT = W @ xT            (64, 2048) per head, head pairs stacked on 128 parts
  v     = x @ WvT           (2048, 256)
  S^T   = k q^T             scores transposed: (s_k, s_q) tiles; diagonal
                            blocks shrunk to the causally-valid q range
  P     = exp(0.125 * S^T)  ScalarE (only exp runs there); the 128-wide
                            triangle at the diagonal masked via DVE mul
  ctx^T = v^T P             accumulated over k tiles in PSUM
  sums  = onescol^T P       quad accumulators at partitions 0/32/64/96
  denominators: DVE evac -> PE head-sum matmul -> DVE reciprocal ->
                GpSimd partition broadcasts -> DVE normalize multiply,
                all riding the next q tile's k loop as fine-grained work
  out   = ctx @ WoT         (2048, 1024) fp32 partial, riding the next loop

All non-attention work (projections of the next q tile, output projection
of the previous one, the normalize chain) is queued as "riders" that are
drained at a fixed cadence inside the attention k loops so PE/DVE/ACT all
stay busy; ScalarE's exp paces the steady state.
"""

import sys

sys.path.insert(0, "/opt/trn_rl_repo")

import numpy as np
import ml_dtypes

import concourse.bass as bass
import concourse.bacc as bacc
import concourse.mybir as mybir
import concourse.tile as tile
from concourse.bass_utils import run_bass_kernel_spmd

BF16 = mybir.dt.bfloat16
F32 = mybir.dt.float32
AF = mybir.ActivationFunctionType

B, S, D, H = 2, 2048, 1024, 16
DH = D // H            # 64
NCORES = 8
NH = 4                 # heads per core
DL = NH * DH           # 256 local head dims per core
KD = D // 128          # 8 contraction chunks for projections
SQ = S // 512          # 4 q tiles of 512
ST = S // 128          # 16 s tiles of 128
SCALE = DH ** -0.5     # 0.125
LAG = 3                # ctx trails scores by LAG k-tiles

_NC = None

TRACE = False
LAST_RESULTS = None
DEBUG = False


def _build_nc():
    nc = bacc.Bacc("TRN2", target_bir_lowering=False, debug=False,
                   num_devices=NCORES)

    xT_d = nc.dram_tensor("xT", [D, S], BF16, kind="ExternalInput")
    wq_d = nc.dram_tensor("wqT", [D, DL], BF16, kind="ExternalInput")
    wk_d = nc.dram_tensor("wkT", [D, DL], BF16, kind="ExternalInput")
    wv_d = nc.dram_tensor("wvT", [D, DL], BF16, kind="ExternalInput")
    wo_d = nc.dram_tensor("woT", [DL, D], BF16, kind="ExternalInput")
    # [128, 2, 128] lower-triangle (q >= k) mask, duplicated per head half
    tri_d = nc.dram_tensor("tri", [128, 256], BF16, kind="ExternalInput")
    # head-sum selector: col h picks quad rows {64*h_parity...}; see below
    sel128_d = nc.dram_tensor("sel128", [128, 2], F32, kind="ExternalInput")
    # ones in column 0, zeros elsewhere (M=32 so quad rows are zero-filled)
    onescol_d = nc.dram_tensor("onescol", [128, 32], BF16, kind="ExternalInput")
    # broadcast selector: row h -> ones at cols [64h, 64h+64)
    sel2_d = nc.dram_tensor("sel2", [2, 128], F32, kind="ExternalInput")
    out_d = nc.dram_tensor("out", [S, D], F32, kind="ExternalOutput")
    dbg = {}
    if DEBUG:
        for j in (1, 2):
            for p in (0, 1):
                dbg[f"q4_{j}_{p}"] = nc.dram_tensor(
                    f"dbg_q4_{j}_{p}", [128, 512], F32, kind="ExternalOutput")
                dbg[f"inv2_{j}_{p}"] = nc.dram_tensor(
                    f"dbg_inv2_{j}_{p}", [2, 512], F32, kind="ExternalOutput")
                dbg[f"invb_{j}_{p}"] = nc.dram_tensor(
                    f"dbg_invb_{j}_{p}", [128, 512], F32, kind="ExternalOutput")
                dbg[f"ctxT_{j}_{p}"] = nc.dram_tensor(
                    f"dbg_ctxT_{j}_{p}", [128, 512], BF16, kind="ExternalOutput")
                dbg[f"ex_{j}_{p}"] = nc.dram_tensor(
                    f"dbg_ex_{j}_{p}", [128, 1024], BF16, kind="ExternalOutput")

    with tile.TileContext(nc) as tc:
        with (
            tc.tile_pool(name="consts", bufs=1) as consts,
            tc.tile_pool(name="xpool", bufs=1) as xpool,
            tc.tile_pool(name="qkpool", bufs=1) as qkpool,
            tc.tile_pool(name="vpool", bufs=1) as vpool,
            tc.tile_pool(name="cpool", bufs=1) as cpool,
            tc.tile_pool(name="exppool", bufs=6) as exppool,
            tc.tile_pool(name="smallpool", bufs=2) as smallpool,
            tc.tile_pool(name="outpool", bufs=6) as outpool,
            tc.tile_pool(name="spsum", bufs=2, space="PSUM") as spsum,
            tc.tile_pool(name="ctxpsum", bufs=2, space="PSUM") as ctxpsum,
            tc.tile_pool(name="mpsum", bufs=1, space="PSUM") as mpsum,
            tc.tile_pool(name="rpsum", bufs=1, space="PSUM") as rpsum,
        ):
            # ---- constants (order matters: q/k weights first, then sq0 x) ----
            wq_sb = consts.tile([128, KD, DL], BF16)
            wk_sb = consts.tile([128, KD, DL], BF16)
            wv_sb = consts.tile([128, KD, DL], BF16)
            wo_sb = consts.tile([128, 2, D], BF16)
            tri_sb = consts.tile([128, 256], BF16)
            sel128_sb = consts.tile([128, 2], F32)
            onescol_sb = consts.tile([128, 32], BF16)
            sel2_sb = consts.tile([2, 128], F32)
            zb = consts.tile([128, 1], F32)

            nc.sync.dma_start(wq_sb[:], wq_d.rearrange("(kd p) j -> p kd j", p=128))
            nc.sync.dma_start(wk_sb[:], wk_d.rearrange("(kd p) j -> p kd j", p=128))

            # ---- persistent activations ----
            xt = [
                [xpool.tile([128, 512], BF16, name=f"xt_{kd}_{sq}",
                            tag=f"xt_{kd}_{sq}") for sq in range(SQ)]
                for kd in range(KD)
            ]
            qT = [
                [qkpool.tile([128, 512], BF16, name=f"qT_{m2}_{sq}",
                             tag=f"qT_{m2}_{sq}") for sq in range(SQ)]
                for m2 in range(2)
            ]
            kT = [
                [qkpool.tile([128, 512], BF16, name=f"kT_{m2}_{sq}",
                             tag=f"kT_{m2}_{sq}") for sq in range(SQ)]
                for m2 in range(2)
            ]
            vt = [
                vpool.tile([128, NH, DH], BF16, name=f"v_{st}", tag=f"v_{st}")
                for st in range(ST)
            ]
            ctxT = [
                [cpool.tile([128, 512], BF16, name=f"ctxT_{kc}_{sq}",
                            tag=f"ctxT_{kc}_{sq}") for sq in range(SQ)]
                for kc in range(2)
            ]

            xT_r = xT_d.rearrange("(kd p) s -> kd p s", p=128)
            # x goes over the (otherwise idle) GpSimd SWDGE queue so the
            # weight loads and output stores on the sync HW queue don't
            # serialize behind it; sq=0 first so projections start early
            for sq in range(SQ):
                for kd in range(KD):
                    nc.gpsimd.dma_start(
                        xt[kd][sq][:], xT_r[kd, :, sq * 512:(sq + 1) * 512]
                    )
            nc.sync.dma_start(tri_sb[:], tri_d[:])
            nc.sync.dma_start(sel128_sb[:], sel128_d[:])
            nc.sync.dma_start(onescol_sb[:], onescol_d[:])
            nc.sync.dma_start(sel2_sb[:], sel2_d[:])
            nc.vector.memset(zb[:], 0.0)
            nc.sync.dma_start(
                wv_sb[:], wv_d.rearrange("(kd p) j -> p kd j", p=128))
            nc.sync.dma_start(
                wo_sb[:], wo_d.rearrange("(kc p) o -> p kc o", p=128))

            # ---- work emitters ----
            def emit_qk_group(sq, which, m2):
                w_sb = wq_sb if which == "q" else wk_sb
                dst = qT if which == "q" else kT
                ps = rpsum.tile([128, 512], F32, name="r_ps", tag="r")
                for kd in range(KD):
                    nc.tensor.matmul(
                        ps[:],
                        w_sb[:, kd, m2 * 128:(m2 + 1) * 128],
                        xt[kd][sq][:],
                        start=(kd == 0),
                        stop=(kd == KD - 1),
                    )
                nc.vector.tensor_copy(dst[m2][sq][:], ps[:])

            def qk_half_riders(sq, which, m2):
                # one q/k projection group split into two riders (4 of the 8
                # contraction chunks each) so a single slot's PE burst stays
                # short; the PSUM accumulator is shared via the closure
                w_sb = wq_sb if which == "q" else wk_sb
                dst = qT if which == "q" else kT
                st = {}

                def h1():
                    ps = rpsum.tile([128, 512], F32, name="r_ps", tag="r")
                    for kd in range(4):
                        nc.tensor.matmul(
                            ps[:],
                            w_sb[:, kd, m2 * 128:(m2 + 1) * 128],
                            xt[kd][sq][:],
                            start=(kd == 0),
                            stop=False,
                        )
                    st["ps"] = ps

                def h2():
                    ps = st["ps"]
                    for kd in range(4, KD):
                        nc.tensor.matmul(
                            ps[:],
                            w_sb[:, kd, m2 * 128:(m2 + 1) * 128],
                            xt[kd][sq][:],
                            start=False,
                            stop=(kd == KD - 1),
                        )
                    nc.vector.tensor_copy(dst[m2][sq][:], ps[:])

                return [h1, h2]

            def emit_v_group(sq, sti):
                st = sq * 4 + sti
                ps = rpsum.tile([128, DL], F32, name="r_ps", tag="r")
                for kd in range(KD):
                    nc.tensor.matmul(
                        ps[:],
                        xt[kd][sq][:, sti * 128:(sti + 1) * 128],
                        wv_sb[:, kd, :],
                        start=(kd == 0),
                        stop=(kd == KD - 1),
                    )
                nc.vector.tensor_copy(
                    vt[st][:].rearrange("p h d -> p (h d)"), ps[:]
                )

            def emit_wo(j, sti, ot):
                st = j * 4 + sti
                o_ps = rpsum.tile([128, 512], F32, name="r_ps", tag="r")
                for kc in range(2):
                    nc.tensor.matmul(
                        o_ps[:],
                        ctxT[kc][j][:, sti * 128:(sti + 1) * 128],
                        wo_sb[:, kc, ot * 512:(ot + 1) * 512],
                        start=(kc == 0),
                        stop=(kc == 1),
                    )
                ob = outpool.tile([128, 512], F32, name="ob", tag="ob")
                nc.vector.tensor_copy(ob[:], o_ps[:])
                nc.sync.dma_start(
                    out_d[st * 128:(st + 1) * 128, ot * 512:(ot + 1) * 512],
                    ob[:],
                )

            # normalize chain state per (j, p): set at loop end, consumed by
            # riders in the following loop
            def make_chain(j, p, q4_sb, c_ps):
                st = {}

                def s_mm1():
                    den2 = rpsum.tile([2, 512], F32, name="den2", tag="r")
                    nc.tensor.matmul(den2[:], sel128_sb[:], q4_sb[:],
                                     start=True, stop=True)
                    st["den2"] = den2

                def s_recip():
                    inv2 = smallpool.tile([2, 512], F32, name="inv2",
                                          tag="inv2")
                    nc.vector.reciprocal_approx_fast(inv2[:], st["den2"][:])
                    st["inv2"] = inv2
                    if DEBUG and f"inv2_{j}_{p}" in dbg:
                        nc.sync.dma_start(dbg[f"inv2_{j}_{p}"][:], inv2[:])

                def s_bcast():
                    inv_ps = rpsum.tile([128, 512], F32, name="inv_ps",
                                        tag="r")
                    nc.tensor.matmul(inv_ps[:], sel2_sb[:], st["inv2"][:],
                                     start=True, stop=True)
                    invb = smallpool.tile([128, 512], F32, name="invb",
                                          tag="invb")
                    nc.vector.tensor_copy(invb[:], inv_ps[:])
                    st["invb"] = invb
                    if DEBUG and f"invb_{j}_{p}" in dbg:
                        nc.sync.dma_start(dbg[f"invb_{j}_{p}"][:], invb[:])

                def s_mul():
                    nc.vector.tensor_mul(ctxT[p][j][:], c_ps[:],
                                         st["invb"][:])
                    if DEBUG and f"ctxT_{j}_{p}" in dbg:
                        nc.sync.dma_start(dbg[f"ctxT_{j}_{p}"][:],
                                          ctxT[p][j][:])

                return [s_mm1, s_recip, s_bcast, s_mul]

            # ---- attention loop for one (j, p) ----
            def attention(j, p, early, bulk, after=[]):
                nkt = 4 * j + 4
                # `early` chain riders run one-per-slot from slot 0 (they
                # recycle the ctx/m PSUM tiles); `bulk` riders (independent of
                # the chain) spread evenly over the loop; `after` riders (the
                # previous q tile's Wo, which READS what the chain writes)
                # must be emitted strictly after the last chain rider.
                E = len(early)
                L = len(bulk)
                A = len(after)
                rem = max(1, nkt - E)
                sched = [
                    ([early[kt]] if kt < E else [])
                    + bulk[(kt * L) // nkt:((kt + 1) * L) // nkt]
                    + (after[((kt - E) * A) // rem:((kt - E + 1) * A) // rem]
                       if kt >= E else [])
                    for kt in range(nkt)
                ]
                c_ps = ctxpsum.tile([128, 512], F32, name="c_ps", tag="ctx")
                m_ps = mpsum.tile([128, 512], F32, name="m_ps", tag="m")
                if j == 0:
                    # odd-kt quad rows start at o=1 (cols 128..512); zero the
                    # never-written head columns so the head-sum matmul reads
                    # defined data
                    nc.vector.memset(m_ps[64:128, 0:128], 0.0)
                exs = {}

                def emit_scores_exp(kt):
                    o = kt - 4 * j
                    q0 = 128 * o if o > 0 else 0
                    s_ps = spsum.tile([128, 1024], F32, name="s_ps", tag="s")
                    for i2 in range(2):
                        hr = i2 * 64
                        nc.tensor.matmul(
                            s_ps[:, i2 * 512 + q0:(i2 + 1) * 512],
                            kT[p][kt // 4][hr:hr + 64,
                                           (kt % 4) * 128:(kt % 4 + 1) * 128],
                            qT[p][j][hr:hr + 64, q0:512],
                            start=True,
                            stop=True,
                        )
                    ex = exppool.tile([128, 1024], BF16, name="ex", tag="ex")
                    if q0 == 0:
                        nc.scalar.activation(
                            ex[:], s_ps[:], AF.Exp, bias=zb[:], scale=SCALE
                        )
                    else:
                        e3 = ex[:].rearrange("p (h q) -> p h q", h=2)
                        s3 = s_ps[:].rearrange("p (h q) -> p h q", h=2)
                        nc.scalar.activation(
                            e3[:, :, q0:512], s3[:, :, q0:512], AF.Exp,
                            bias=zb[:], scale=SCALE
                        )
                    if o >= 0:
                        # triangle mask on the 128-wide diagonal window
                        e3 = ex[:].rearrange("p (h q) -> p h q", h=2)
                        t3 = tri_sb[:].rearrange("p (h q) -> p h q", h=2)
                        nc.vector.tensor_mul(
                            e3[:, :, q0:q0 + 128], e3[:, :, q0:q0 + 128],
                            t3[:]
                        )
                    exs[kt] = ex
                    if DEBUG and kt == 4 * j + 1 and f"ex_{j}_{p}" in dbg:
                        nc.sync.dma_start(dbg[f"ex_{j}_{p}"][:], ex[:])

                def emit_ctx(kt):
                    o = kt - 4 * j
                    q0 = 128 * o if o > 0 else 0
                    ex = exs[kt]
                    e3 = ex[:].rearrange("p (h q) -> p h q", h=2)
                    for i2 in range(2):
                        nc.tensor.matmul(
                            c_ps[64 * i2:64 * i2 + DH, q0:512],
                            vt[kt][:, 2 * p + i2, :],
                            e3[:, i2, q0:512],
                            start=(kt == 0),
                            stop=(kt == nkt - 1),
                            tile_position=(0, 64 * i2),
                        )
                    if kt % 2 == 1:
                        op = kt - 1 - 4 * j
                        qp = 128 * op if op > 0 else 0
                        exprev = exs.pop(kt - 1)
                        ep3 = exprev[:].rearrange("p (h q) -> p h q", h=2)
                        for idx, (eq3, i2, qq0) in enumerate(
                            [(ep3, 0, qp), (ep3, 1, qp),
                             (e3, 0, q0), (e3, 1, q0)]
                        ):
                            pos = 32 * idx
                            nc.tensor.matmul(
                                m_ps[pos:pos + 32, qq0:512],
                                onescol_sb[:],
                                eq3[:, i2, qq0:512],
                                start=(kt == 1),
                                stop=(kt == nkt - 1),
                                tile_position=(0, pos),
                            )

                for kt in range(nkt):
                    emit_scores_exp(kt)
                    if kt >= LAG:
                        emit_ctx(kt - LAG)
                    for r in sched[kt]:
                        r()
                for kt in range(max(0, nkt - LAG), nkt):
                    emit_ctx(kt)
                exs.clear()

                # evacuate the quad sums promptly (frees m_ps for the next
                # loop); the rest of the chain rides the next loop
                q4_sb = smallpool.tile([128, 512], F32, name="q4", tag="q4")
                nc.vector.tensor_copy(q4_sb[:], m_ps[:])
                if DEBUG and f"q4_{j}_{p}" in dbg:
                    nc.sync.dma_start(dbg[f"q4_{j}_{p}"][:], q4_sb[:])
                return make_chain(j, p, q4_sb, c_ps)

            # ---- prologue: just enough to start (j=0, p=0) ----
            emit_qk_group(0, "q", 0)
            emit_qk_group(0, "k", 0)
            emit_v_group(0, 0)
            emit_v_group(0, 1)

            def qkv_riders(sq):
                return [
                    lambda s=sq: emit_qk_group(s, "q", 0),
                    lambda s=sq: emit_qk_group(s, "k", 0),
                    lambda s=sq: emit_qk_group(s, "q", 1),
                    lambda s=sq: emit_qk_group(s, "k", 1),
                    lambda s=sq: emit_v_group(s, 0),
                    lambda s=sq: emit_v_group(s, 1),
                    lambda s=sq: emit_v_group(s, 2),
                    lambda s=sq: emit_v_group(s, 3),
                ]

            def wo_riders(j):
                return [
                    (lambda jj=j, s=sti, o=ot: emit_wo(jj, s, o))
                    for sti in range(4) for ot in range(2)
                ]

            # ---- main loops ----
            chain = {}
            chain[(0, 0)] = attention(
                0, 0, [],
                [lambda: emit_qk_group(0, "q", 1),
                 lambda: emit_qk_group(0, "k", 1),
                 lambda: emit_v_group(0, 2),
                 lambda: emit_v_group(0, 3)],
            )
            chain[(0, 1)] = attention(0, 1, chain[(0, 0)], qkv_riders(1))
            # sq=3's k/v projections ride j=3's own (ScalarE-bound) loops —
            # they are only consumed from kt=12 — so PE-bound j=2 stays lean
            bulk_j = {
                1: qkv_riders(2),
                2: [lambda: emit_qk_group(3, "q", 0),
                    lambda: emit_qk_group(3, "q", 1)],
                3: ([lambda: emit_qk_group(3, "k", 0),
                     lambda: emit_qk_group(3, "k", 1)]
                    + [lambda t=sti: emit_v_group(3, t) for sti in range(4)]),
            }
            for j in range(1, SQ):
                chain[(j, 0)] = attention(j, 0, chain[(j - 1, 1)], bulk_j[j],
                                          after=wo_riders(j - 1))
                chain[(j, 1)] = attention(j, 1, chain[(j, 0)], [])

            # ---- epilogue ----
            for step in chain[(SQ - 1, 1)]:
                step()
            for sti in range(4):
                for ot in range(2):
                    emit_wo(SQ - 1, sti, ot)

    nc.compile()
    return nc


def _get_nc():
    global _NC
    if _NC is None:
        _NC = _build_nc()
    return _NC


def _bf16(a):
    return np.ascontiguousarray(a).astype(ml_dtypes.bfloat16)


def kernel(x, Wq, Wk, Wv, Wo, bo):
    global LAST_RESULTS
    x = np.asarray(x, dtype=np.float32)
    Wq = np.asarray(Wq, dtype=np.float32)
    Wk = np.asarray(Wk, dtype=np.float32)
    Wv = np.asarray(Wv, dtype=np.float32)
    Wo = np.asarray(Wo, dtype=np.float32)
    bo = np.asarray(bo, dtype=np.float32)

    xT = [_bf16(x[b].T) for b in range(B)]          # (D, S)
    WqT = np.ascontiguousarray(Wq.T)                # (D, D): col slice = head rows
    WkT = np.ascontiguousarray(Wk.T)
    WvT = np.ascontiguousarray(Wv.T)
    WoT = np.ascontiguousarray(Wo.T)                # (D, D): row slice = ctx dims

    kk = np.arange(128)[:, None]
    cc = np.arange(128)[None, :]
    tri1 = (cc >= kk).astype(np.float32)            # (128, 128)
    tri = np.concatenate([tri1, tri1], axis=1).astype(ml_dtypes.bfloat16)

    sel128 = np.zeros((128, 2), dtype=np.float32)
    sel128[0, 0] = sel128[64, 0] = 1.0              # head 0: even+odd rows
    sel128[32, 1] = sel128[96, 1] = 1.0             # head 1
    onescol = np.zeros((128, 32), dtype=np.float32)
    onescol[:, 0] = 1.0
    onescol = onescol.astype(ml_dtypes.bfloat16)
    sel2 = np.zeros((2, 128), dtype=np.float32)
    sel2[0, 0:64] = 1.0
    sel2[1, 64:128] = 1.0

    in_maps = []
    for c in range(NCORES):
        b, g = divmod(c, 4)
        sl = slice(g * DL, (g + 1) * DL)
        in_maps.append(
            {
                "xT": xT[b],
                "wqT": _bf16(WqT[:, sl]),
                "wkT": _bf16(WkT[:, sl]),
                "wvT": _bf16(WvT[:, sl]),
                "woT": _bf16(WoT[sl, :]),
                "tri": tri,
                "sel128": sel128,
                "onescol": onescol,
                "sel2": sel2,
            }
        )

    nc = _get_nc()
    results = run_bass_kernel_spmd(
        nc, in_maps, core_ids=list(range(NCORES)), trace=TRACE
    )
    LAST_RESULTS = results

    out = np.zeros((B, S, D), dtype=np.float32)
    for c in range(NCORES):
        out[c // 4] += results.results[c]["out"]
    out += bo[None, None, :]
    return out
